# revision 1
# baseline (speedup 1.0000x reference)
"""GMM log-likelihood kernel for Trainium2 (Bass/Tile), 8-core data-parallel.

Math (host precompute in f64):
  B_k = L_k^{-1} (Cholesky inverse),  w_k = B_k^T B_k mu_k
  wlp_k(x) = -0.5*||B_k x||^2 + w_k . x + C_k
  lse(x)   = m0 + log(sum_k exp(wlp_k - m0))   (m0 = global shift, safe:
             measured per-sample max wlp spread is ~37 nats << f32 exp range)
  out      = sum_x lse(x)

Per core: the [25000, 64] data slice (zero-padded to 196 tiles of 128
samples) is processed in pairs of tiles: PE transposes each pair into a
[128,128] stationary (two 64-row feature blocks), then row-packed bf16
matmuls against the replicated moving operand [B_all | W] produce
Y [128 samples, 1024] + lin [128, 16] per tile.  ACT squares Y out of
PSUM, DVE group-reduces the squares to per-component norms and assembles
wlp into a [128, 196*16] buffer.  A batched phase 2 does exp /
component-sum / log / masked accumulate, and a ones-matmul folds the 128
partitions into the final scalar.  Host sums the 8 per-core scalars.
"""

import numpy as np

N_COMPONENTS = 16
N_FEATURES = 64
N_SAMPLES = 200000
N_CORES = 8
PER_CORE = N_SAMPLES // N_CORES          # 25000
TILE_P = 128
N_TILES = -(-PER_CORE // TILE_P)         # 196 (ceil)
N_PAIRS = (N_TILES + 1) // 2             # 98
PADDED = N_TILES * TILE_P                # 25088
KD = N_COMPONENTS * N_FEATURES           # 1024

_CACHE = {}


def _build_nc(n_pairs):
    import concourse.tile as tile
    from concourse import bacc, mybir

    n_tiles = n_pairs * 2
    padded = n_tiles * TILE_P
    f32 = mybir.dt.float32
    bf16 = mybir.dt.bfloat16

    nc = bacc.Bacc("TRN2", target_bir_lowering=False, debug=False,
                   num_devices=N_CORES)

    xp = nc.dram_tensor("xp", [padded, N_FEATURES], bf16, kind="ExternalInput").ap()
    bmov2 = nc.dram_tensor("bmov2", [128, KD + N_COMPONENTS], bf16,
                           kind="ExternalInput").ap()
    cq = nc.dram_tensor("cq", [1, N_COMPONENTS], f32, kind="ExternalInput").ap()
    oner = nc.dram_tensor("oner", [1, 128], f32, kind="ExternalInput").ap()
    mask = nc.dram_tensor("mask", [128, n_tiles], f32, kind="ExternalInput").ap()
    ident = nc.dram_tensor("ident", [128, 128], bf16, kind="ExternalInput").ap()
    ones = nc.dram_tensor("ones", [128, 1], f32, kind="ExternalInput").ap()
    out = nc.dram_tensor("out", [1, 1], f32, kind="ExternalOutput").ap()

    W = n_tiles * N_COMPONENTS

    with tile.TileContext(nc) as tc:
        with (
            tc.tile_pool(name="const", bufs=1) as const_pool,
            tc.tile_pool(name="wbuf", bufs=1) as wbuf_pool,
            tc.tile_pool(name="xin", bufs=4) as xin_pool,
            tc.tile_pool(name="xt", bufs=3) as xt_pool,
            tc.tile_pool(name="ysq", bufs=2) as ysq_pool,
            tc.tile_pool(name="sm", bufs=4) as sm_pool,
            tc.tile_pool(name="tp", bufs=2, space="PSUM") as tp_pool,
            tc.tile_pool(name="yp", bufs=2, space="PSUM") as yp_pool,
            tc.tile_pool(name="lp", bufs=2, space="PSUM") as lp_pool,
        ):
            bm = const_pool.tile([128, KD + N_COMPONENTS], bf16)
            nc.sync.dma_start(bm[:], bmov2[:])
            cqs = const_pool.tile([1, N_COMPONENTS], f32)
            nc.sync.dma_start(cqs[:], cq[:])
            onr = const_pool.tile([1, 128], f32)
            nc.sync.dma_start(onr[:], oner[:])
            msks = const_pool.tile([128, n_tiles], f32)
            nc.sync.dma_start(msks[:], mask[:])
            idn = const_pool.tile([128, 128], bf16)
            nc.sync.dma_start(idn[:], ident[:])
            on1 = const_pool.tile([128, 1], f32)
            nc.sync.dma_start(on1[:], ones[:])

            wbuf = wbuf_pool.tile([128, W], f32)
            ebuf = wbuf_pool.tile([128, W], f32)

            for p in range(n_pairs):
                xpair = xin_pool.tile([128, 128], bf16, tag="xpair")
                r0 = (2 * p) * TILE_P
                nc.sync.dma_start(xpair[:, 0:64], xp[r0:r0 + 128, :])
                nc.sync.dma_start(xpair[:, 64:128], xp[r0 + 128:r0 + 256, :])

                tp = tp_pool.tile([128, 128], bf16, tag="tp")
                nc.tensor.transpose(tp[:], xpair[:], idn[:])
                xt = xt_pool.tile([128, 128], bf16, tag="xt")
                nc.scalar.copy(xt[:], tp[:])

                ysq = ysq_pool.tile([128, 2 * KD], f32, tag="ysq")
                lps = []
                for h in range(2):
                    hp = h * 64
                    yp = yp_pool.tile([128, KD], f32, tag="yp")
                    lp = lp_pool.tile([128, N_COMPONENTS], f32, tag="lp")
                    lhs = xt[hp:hp + 64, :]
                    nc.tensor.matmul(yp[:, 0:512], lhs, bm[hp:hp + 64, 0:512])
                    nc.tensor.matmul(yp[:, 512:1024], lhs, bm[hp:hp + 64, 512:1024])
                    nc.tensor.matmul(lp[:], lhs, bm[hp:hp + 64, 1024:1040],
                                     start=True, stop=False)
                    nc.tensor.matmul(lp[:], onr[:], cqs[:],
                                     start=False, stop=True)
                    nc.scalar.activation(ysq[:, h * KD:(h + 1) * KD], yp[:],
                                         mybir.ActivationFunctionType.Square)
                    lps.append(lp)

                st = sm_pool.tile([128, 2 * N_COMPONENTS], f32, tag="st")
                nc.vector.reduce_sum(
                    st[:],
                    ysq[:].rearrange("p (k i) -> p k i", i=N_FEATURES),
                    axis=mybir.AxisListType.X)

                for h in range(2):
                    col = (2 * p + h) * N_COMPONENTS
                    nc.vector.scalar_tensor_tensor(
                        wbuf[:, col:col + N_COMPONENTS],
                        st[:, h * N_COMPONENTS:(h + 1) * N_COMPONENTS],
                        -0.5, lps[h][:],
                        op0=mybir.AluOpType.mult, op1=mybir.AluOpType.add)

            # phase 2
            nc.scalar.activation(ebuf[:], wbuf[:],
                                 mybir.ActivationFunctionType.Exp)
            rsum = const_pool.tile([128, n_tiles], f32)
            nc.vector.reduce_sum(
                rsum[:],
                ebuf[:].rearrange("p (t k) -> p t k", k=N_COMPONENTS),
                axis=mybir.AxisListType.X)
            lnr = const_pool.tile([128, n_tiles], f32)
            nc.scalar.activation(lnr[:], rsum[:],
                                 mybir.ActivationFunctionType.Ln)
            msum = const_pool.tile([128, n_tiles], f32)
            nc.vector.tensor_mul(msum[:], lnr[:], msks[:])
            csum = const_pool.tile([128, 1], f32)
            nc.vector.reduce_sum(csum[:], msum[:], axis=mybir.AxisListType.X)

            rp = tp_pool.tile([1, 1], f32, tag="tp")
            nc.tensor.matmul(rp[:], on1[:], csum[:])
            res = const_pool.tile([1, 1], f32)
            nc.scalar.copy(res[:], rp[:])
            nc.sync.dma_start(out[:], res[:])

    nc.compile()
    return nc


def _precompute(weights, means, covariances):
    """Host-side O(K d^3) prep in float64. Returns (bmov2, cq_row, m0)."""
    import ml_dtypes

    K, d = means.shape
    L = np.linalg.cholesky(covariances.astype(np.float64))
    half_logdet = np.log(np.diagonal(L, axis1=-2, axis2=-1)).sum(-1)
    eye = np.eye(d)
    B = np.stack([np.linalg.solve(L[k], eye) for k in range(K)])  # L^-1
    mu = means.astype(np.float64)
    c = np.einsum('kij,kj->ki', B, mu)
    w_lin = np.einsum('kij,ki->kj', B, c)
    r = (c * c).sum(-1)
    const = (np.log(weights.astype(np.float64))
             - 0.5 * d * np.log(2.0 * np.pi) - half_logdet)
    C = const - 0.5 * r
    m0 = float(C.max()) - 20.0

    bmov = np.zeros((d, K * d + K), np.float32)
    for k in range(K):
        bmov[:, k * d:(k + 1) * d] = B[k].T.astype(np.float32)
    bmov[:, K * d:] = w_lin.T.astype(np.float32)
    bmov2 = np.vstack([bmov, bmov]).astype(ml_dtypes.bfloat16)   # [128, 1040]
    cq_row = (C - m0).astype(np.float32)                         # [16]
    return bmov2, cq_row, m0


def _make_inputs(data, bmov2, cq_row, n_tiles):
    """Build the 8 per-core input maps for the padded per-core data slices."""
    import ml_dtypes

    padded = n_tiles * TILE_P
    cq = cq_row[None, :].astype(np.float32)
    oner = np.ones((1, 128), np.float32)
    mask = np.zeros((128, n_tiles), np.float32)
    for t in range(n_tiles):
        v = min(max(PER_CORE - t * TILE_P, 0), TILE_P)
        mask[:v, t] = 1.0
    ident = np.eye(128, dtype=ml_dtypes.bfloat16)
    ones = np.ones((128, 1), np.float32)

    in_maps = []
    for c in range(N_CORES):
        sl = data[c * PER_CORE:(c + 1) * PER_CORE]
        xp = np.zeros((padded, N_FEATURES), ml_dtypes.bfloat16)
        xp[:sl.shape[0]] = sl.astype(ml_dtypes.bfloat16)
        in_maps.append({"xp": xp, "bmov2": bmov2, "cq": cq, "mask": mask,
                        "ident": ident, "ones": ones, "oner": oner})
    return in_maps


def _run(data, weights, means, covariances, trace=False):
    from concourse.bass_utils import run_bass_kernel_spmd

    data = np.asarray(data, np.float32)
    bmov2, cq_row, m0 = _precompute(np.asarray(weights), np.asarray(means),
                                    np.asarray(covariances))
    if "nc" not in _CACHE:
        _CACHE["nc"] = _build_nc(N_PAIRS)
    nc = _CACHE["nc"]

    in_maps = _make_inputs(data, bmov2, cq_row, N_TILES)
    res = run_bass_kernel_spmd(nc, in_maps, list(range(N_CORES)), trace=trace)
    total = 0.0
    for c in range(N_CORES):
        total += float(res.results[c]["out"][0, 0]) + PER_CORE * m0
    return np.float32(total), res


def kernel(data, weights, means, covariances):
    return _run(data, weights, means, covariances)[0]

